# revision 13
# baseline (speedup 1.0000x reference)
"""Trainium2 Bass kernel for nn_BlockSparseMLP (MoE gated MLP, E=8, top-2).

Strategy: expert parallelism over 8 NeuronCores. The router matmul
(x @ w_router, 67 MFLOP out of the 206 GFLOP total) plus the top-2
dispatch/gather and the final scatter-add combine run on the host; each
core runs the full gated MLP (gate/up, silu*up, down, weighted by the
routing prob) for the tokens routed to its expert. Matmuls use float32r
(full-rate fp32 mode on the PE, ~tf32 accuracy), activations stay fp32.

Per-core device layout (capacity C = 512 + 128*nb1 tokens, token-major
tiles of 128):
  phase 1 (gate/up): stream w_gate/w_up in [128, 256] chunks; block-0
    (512 tokens) uses weights as stationary and xT as moving (N=512);
    block-1 (the 128*nb1 overflow tokens) uses tokens as stationary and
    the weight chunk as moving (N=256), then PE-transposes the result
    into the [I, tokens] layout. silu(gate)*up is fused on ACT+DVE into
    aT ([I, C] layout, resident in SBUF).
  phase 2 (down): stream w_down in [128, 512] chunks (moving), aT tiles
    stationary, accumulate over I into [tokens, 512] psum tiles, scale
    by the per-token routing weight, DMA out.
"""

import sys
import functools

sys.path.insert(0, "/opt/trn_rl_repo")

import numpy as np

T, H, II, E, TOPK = 2048, 2048, 4096, 8, 2
NCORES = 8
B0 = 512        # block-0 token capacity (moving N for gate/up)
CHUNK = 256     # phase-1 weight chunk width along I
KT = H // 128   # 16 contraction tiles for gate/up
MTI = II // 128  # 32 I tiles
NMC = II // CHUNK  # 16 weight chunks
JJ = CHUNK // 128  # 2 m-tiles per chunk
KI = II // 128  # 32 contraction tiles for down
NH = H // 512   # 4 output column chunks


@functools.lru_cache(maxsize=2)
def _build(nb1: int):
    """Build the SPMD Bass program for capacity 512 + 128*nb1 tokens."""
    import concourse.mybir as mybir
    import concourse.tile as tile
    from concourse import bacc
    from concourse.masks import make_identity

    f32 = mybir.dt.float32
    f32r = mybir.dt.float32r
    NT = B0 // 128 + nb1

    nc = bacc.Bacc(None)
    xT0 = nc.declare_dram_parameter("xT0", [KT, 128, B0], f32r, isOutput=False)
    if nb1:
        xT1 = nc.declare_dram_parameter("xT1", [KT, 128, 128 * nb1], f32r, isOutput=False)
    wg = nc.declare_dram_parameter("wg", [H, II], f32r, isOutput=False)
    wu = nc.declare_dram_parameter("wu", [H, II], f32r, isOutput=False)
    wd = nc.declare_dram_parameter("wd", [II, H], f32r, isOutput=False)
    rw = nc.declare_dram_parameter("rw", [128, NT], f32, isOutput=False)
    dout = nc.declare_dram_parameter("d", [NT, 128, H], f32, isOutput=True)

    SILU = mybir.ActivationFunctionType.Silu

    with tile.TileContext(nc) as tc:
        with (
            tc.tile_pool(name="pers", bufs=1) as pers,
            tc.tile_pool(name="wpool", bufs=20) as wpool,
        ):
            aT0 = pers.tile([128, MTI, B0], f32r)
            if nb1:
                aT1 = pers.tile([128, nb1, MTI, 128], f32r)
                sg1 = pers.tile([128, nb1, NMC, CHUNK], f32)
                ident = pers.tile([128, 128], f32)
                make_identity(nc, ident[:])
            rwt = pers.tile([128, NT], f32)
            nc.gpsimd.dma_start(rwt[:], rw[:])

            with (
                tc.tile_pool(name="xp", bufs=1) as xp,
                tc.tile_pool(name="ps1", bufs=1, space="PSUM") as ps1,
                tc.tile_pool(name="sp", bufs=2) as sp,
            ):
                xt0 = []
                xt1 = []
                for k in range(KT):
                    eng = nc.sync if k % 2 == 0 else nc.scalar
                    t0 = xp.tile([128, B0], f32r, name=f"xt0_{k}")
                    eng.dma_start(t0[:], xT0[k])
                    xt0.append(t0)
                    if nb1:
                        t1 = xp.tile([128, 128 * nb1], f32r, name=f"xt1_{k}")
                        eng.dma_start(t1[:], xT1[k])
                        xt1.append(t1)

                for mc in range(NMC):
                    # One [128, 512] tile holds this chunk of BOTH w_gate
                    # (cols 0:256) and w_up (cols 256:512): block-1 then
                    # computes gate and up in a single full-rate N=512 matmul.
                    wb = []
                    for k in range(KT):
                        wbk = wpool.tile([128, 2 * CHUNK], f32r, name="wbk", tag="w")
                        nc.sync.dma_start(
                            wbk[:, :CHUNK],
                            wg[k * 128:(k + 1) * 128, mc * CHUNK:(mc + 1) * CHUNK],
                        )
                        nc.scalar.dma_start(
                            wbk[:, CHUNK:],
                            wu[k * 128:(k + 1) * 128, mc * CHUNK:(mc + 1) * CHUNK],
                        )
                        wb.append(wbk)

                    pg0 = [ps1.tile([128, B0], f32, name="pg0", tag="pg0", bufs=3)
                           for _ in range(JJ)]
                    pu0 = [ps1.tile([128, B0], f32, name="pu0", tag="pu0", bufs=3)
                           for _ in range(JJ)]
                    pgu1 = [ps1.tile([128, 2 * CHUNK], f32, name="pgu1", tag="pgu1", bufs=2)
                            for _ in range(nb1)]

                    for k in range(KT):
                        st = dict(start=(k == 0), stop=(k == KT - 1))
                        for j in range(JJ):
                            nc.tensor.matmul(
                                pg0[j][:], wb[k][:, j * 128:(j + 1) * 128], xt0[k][:], **st
                            )
                        for bt in range(nb1):
                            nc.tensor.matmul(
                                pgu1[bt][:], xt1[k][:, bt * 128:(bt + 1) * 128], wb[k][:], **st
                            )
                        for j in range(JJ):
                            nc.tensor.matmul(
                                pu0[j][:],
                                wb[k][:, CHUNK + j * 128:CHUNK + (j + 1) * 128],
                                xt0[k][:], **st
                            )

                    for j in range(JJ):
                        m = mc * JJ + j
                        sg = sp.tile([128, B0], f32, name="sg", tag="sg")
                        nc.scalar.activation(sg[:], pg0[j][:], SILU)
                        nc.vector.tensor_mul(aT0[:, m, :], sg[:], pu0[j][:])
                    for bt in range(nb1):
                        s1 = sg1[:, bt, mc, :]
                        nc.scalar.activation(s1, pgu1[bt][:, :CHUNK], SILU)
                        nc.vector.tensor_mul(s1, s1, pgu1[bt][:, CHUNK:])

            with (
                tc.tile_pool(name="ps2", bufs=1, space="PSUM") as ps2,
                tc.tile_pool(name="op", bufs=3) as op,
            ):
                pd_bufs = min(8, NT + 3)
                # Transpose the block-1 activations into [I, tokens] layout at
                # the head of phase 2 (the PE is otherwise idle while w_down
                # prefetch starts); copies alternate DVE/ACT to keep up.
                for bt in range(nb1):
                    for mi in range(MTI):
                        mc, t = divmod(mi, JJ)
                        ptr = ps2.tile([128, 128], f32, name="ptr", tag="pd", bufs=pd_bufs)
                        nc.tensor.transpose(
                            ptr[:], sg1[:, bt, mc, t * 128:(t + 1) * 128], ident[:]
                        )
                        if mi % 2 == 0:
                            nc.vector.tensor_copy(aT1[:, bt, mi, :], ptr[:])
                        else:
                            nc.scalar.copy(aT1[:, bt, mi, :], ptr[:])
                for nh in range(NH):
                    pd = [ps2.tile([128, 512], f32, name="pd", tag="pd", bufs=pd_bufs)
                          for _ in range(NT)]
                    for k in range(KI):
                        wdk = wpool.tile([128, 512], f32r, name="wdk", tag="w")
                        eng = nc.sync if k % 2 == 0 else nc.scalar
                        eng.dma_start(
                            wdk[:], wd[k * 128:(k + 1) * 128, nh * 512:(nh + 1) * 512]
                        )
                        st = dict(start=(k == 0), stop=(k == KI - 1))
                        for mt in range(NT):
                            if mt < B0 // 128:
                                lhsT = aT0[:, k, mt * 128:(mt + 1) * 128]
                            else:
                                lhsT = aT1[:, mt - B0 // 128, k, :]
                            nc.tensor.matmul(pd[mt][:], lhsT, wdk[:], **st)
                    for mt in range(NT):
                        ot = op.tile([128, 512], f32, name="ot", tag="ot")
                        nc.vector.tensor_scalar_mul(ot[:], pd[mt][:], rwt[:, mt:mt + 1])
                        nc.gpsimd.dma_start(dout[mt][:, nh * 512:(nh + 1) * 512], ot[:])

    nc.compile()
    return nc


@functools.lru_cache(maxsize=2)
def _get_exec(nb1: int):
    """Compile the Bass program and return (nc, run_fn) with a cached jit.

    run_fn(in_maps) -> list of per-core {"d": np.ndarray}. Mirrors
    bass2jax.run_bass_via_pjrt's multi-core branch, but keeps the jitted
    function alive across kernel() calls so repeat invocations skip XLA
    and NEFF compilation.
    """
    import jax
    import concourse.mybir as mybir
    from concourse import bass2jax

    nc = _build(nb1)
    bass2jax.install_neuronx_cc_hook()

    partition_name = nc.partition_id_tensor.name if nc.partition_id_tensor else None
    in_names, out_names, out_avals = [], [], []
    zero_out_shapes = []
    for alloc in nc.m.functions[0].allocations:
        if not isinstance(alloc, mybir.MemoryLocationSet):
            continue
        name = alloc.memorylocations[0].name
        if alloc.kind == "ExternalInput":
            if name != partition_name:
                in_names.append(name)
        elif alloc.kind == "ExternalOutput":
            shape = tuple(alloc.tensor_shape)
            dtype = mybir.dt.np(alloc.dtype)
            out_names.append(name)
            out_avals.append(jax.core.ShapedArray(shape, dtype))
            zero_out_shapes.append((shape, dtype))
    n_params = len(in_names)
    n_outs = len(out_names)
    all_names = list(in_names) + list(out_names)
    if partition_name is not None:
        all_names.append(partition_name)
    donate = tuple(range(n_params, n_params + n_outs))

    def _body(*args):
        operands = list(args)
        if partition_name is not None:
            operands.append(bass2jax.partition_id_tensor())
        outs = bass2jax._bass_exec_p.bind(
            *operands,
            out_avals=tuple(out_avals),
            in_names=tuple(all_names),
            out_names=tuple(out_names),
            lowering_input_output_aliases=(),
            sim_require_finite=True,
            sim_require_nnan=True,
            nc=nc,
        )
        return tuple(outs)

    devices = jax.devices()[:NCORES]
    assert len(devices) == NCORES, f"need {NCORES} devices, have {len(jax.devices())}"
    mesh = bass2jax.Mesh(np.asarray(devices), ("core",))
    in_specs = (bass2jax.PartitionSpec("core"),) * (n_params + n_outs)
    out_specs = (bass2jax.PartitionSpec("core"),) * n_outs
    sharded = jax.jit(
        bass2jax.shard_map(
            _body, mesh=mesh, in_specs=in_specs, out_specs=out_specs, check_rep=False
        ),
        donate_argnums=donate,
        keep_unused=True,
    )

    def run_fn(in_maps):
        concat_in = [
            np.concatenate([np.asarray(m[name]) for m in in_maps], axis=0)
            for name in in_names
        ]
        zeros = [
            np.zeros((shape[0] * NCORES,) + shape[1:], dtype)
            for shape, dtype in zero_out_shapes
        ]
        out_arrs = sharded(*concat_in, *zeros)
        results = []
        for c in range(NCORES):
            res = {}
            for i, name in enumerate(out_names):
                arr = np.asarray(out_arrs[i])
                per = arr.shape[0] // NCORES
                res[name] = arr[c * per:(c + 1) * per]
            results.append(res)
        return results

    return nc, run_fn


def _route(x, w_router):
    """Top-2 routing: expert ids + softmax weights, matching jax.lax.top_k
    (descending, ties to the lower index) + jax.nn.softmax."""
    logits = x.astype(np.float64) @ w_router.astype(np.float64)
    top2 = np.argsort(-logits, axis=1, kind="stable")[:, :TOPK]
    vals = np.take_along_axis(logits, top2, 1).astype(np.float32)
    e = np.exp(vals - vals.max(axis=1, keepdims=True))
    w = (e / e.sum(axis=1, keepdims=True)).astype(np.float32)
    return top2, w


def _reference_numpy(x, w_router, w_gate, w_up, w_down):
    """Correct-but-slow dense fallback for shapes the device program doesn't cover."""
    x = x.astype(np.float32)
    logits = x @ w_router.astype(np.float32)
    n_exp = w_gate.shape[0]
    k = min(TOPK, n_exp)
    top = np.argsort(-logits, axis=1, kind="stable")[:, :k]
    vals = np.take_along_axis(logits, top, 1)
    ex = np.exp(vals - vals.max(1, keepdims=True))
    ww = (ex / ex.sum(1, keepdims=True)).astype(np.float32)
    w_dense = np.zeros_like(logits)
    t_ids = np.arange(x.shape[0])[:, None]
    w_dense[t_ids, top] = ww
    out = np.zeros((x.shape[0], w_down.shape[-1]), np.float32)
    for e in range(n_exp):
        g = x @ w_gate[e]
        u = x @ w_up[e]
        a = (g / (1.0 + np.exp(-g))) * u
        out += w_dense[:, e:e + 1] * (a @ w_down[e])
    return out


def kernel(x, w_router, w_gate, w_up, w_down):
    x = np.ascontiguousarray(np.asarray(x, dtype=np.float32))
    w_router = np.asarray(w_router, dtype=np.float32)
    w_gate = np.ascontiguousarray(np.asarray(w_gate, dtype=np.float32))
    w_up = np.ascontiguousarray(np.asarray(w_up, dtype=np.float32))
    w_down = np.ascontiguousarray(np.asarray(w_down, dtype=np.float32))

    if (x.shape != (T, H) or w_router.shape != (H, E)
            or w_gate.shape != (E, H, II) or w_up.shape != (E, H, II)
            or w_down.shape != (E, II, H)):
        return _reference_numpy(x, w_router, w_gate, w_up, w_down)

    top2, w = _route(x, w_router)
    tok = np.repeat(np.arange(T), TOPK)
    te = top2.ravel()
    tw = w.ravel()
    toks_e, ws_e = [], []
    for e in range(E):
        sel = te == e
        toks_e.append(tok[sel])
        ws_e.append(tw[sel].astype(np.float32))

    max_n = max(len(t) for t in toks_e)
    nb1 = max(0, -(-(max_n - B0) // 128))  # ceil((max_n - 512)/128), >= 0
    if nb1 > 2:
        return _reference_numpy(x, w_router, w_gate, w_up, w_down)

    C = B0 + 128 * nb1
    NT = C // 128

    nc, run_fn = _get_exec(nb1)

    in_maps = []
    for e in range(E):
        n_e = len(toks_e[e])
        xg = np.zeros((C, H), np.float32)
        xg[:n_e] = x[toks_e[e]]
        xT = np.ascontiguousarray(xg.T).reshape(KT, 128, C)
        rfull = np.zeros(C, np.float32)
        rfull[:n_e] = ws_e[e]
        m = {
            "xT0": np.ascontiguousarray(xT[:, :, :B0]),
            "wg": w_gate[e],
            "wu": w_up[e],
            "wd": w_down[e],
            "rw": np.ascontiguousarray(rfull.reshape(NT, 128).T),
        }
        if nb1:
            m["xT1"] = np.ascontiguousarray(xT[:, :, B0:])
        in_maps.append(m)

    results = run_fn(in_maps)

    out = np.zeros((T, H), np.float32)
    for e in range(E):
        n_e = len(toks_e[e])
        d = results[e]["d"].reshape(C, H)
        out[toks_e[e]] += d[:n_e]
    return out


# revision 14
# speedup vs baseline: 1.0121x; 1.0121x over previous
"""Trainium2 Bass kernel for nn_BlockSparseMLP (MoE gated MLP, E=8, top-2).

Strategy: expert parallelism over 8 NeuronCores. The router matmul
(x @ w_router, 67 MFLOP out of the 206 GFLOP total) plus the top-2
dispatch/gather and the final scatter-add combine run on the host; each
core runs the full gated MLP (gate/up, silu*up, down, weighted by the
routing prob) for the tokens routed to its expert. Matmuls use float32r
(full-rate fp32 mode on the PE, ~tf32 accuracy), activations stay fp32.

Per-core device layout (capacity C = 512 + 128*nb1 tokens, token-major
tiles of 128):
  phase 1 (gate/up): stream w_gate/w_up in [128, 256] chunks; block-0
    (512 tokens) uses weights as stationary and xT as moving (N=512);
    block-1 (the 128*nb1 overflow tokens) uses tokens as stationary and
    the weight chunk as moving (N=256), then PE-transposes the result
    into the [I, tokens] layout. silu(gate)*up is fused on ACT+DVE into
    aT ([I, C] layout, resident in SBUF).
  phase 2 (down): stream w_down in [128, 512] chunks (moving), aT tiles
    stationary, accumulate over I into [tokens, 512] psum tiles, scale
    by the per-token routing weight, DMA out.
"""

import sys
import functools

sys.path.insert(0, "/opt/trn_rl_repo")

import numpy as np

T, H, II, E, TOPK = 2048, 2048, 4096, 8, 2
NCORES = 8
B0 = 512        # block-0 token capacity (moving N for gate/up)
CHUNK = 256     # phase-1 weight chunk width along I
KT = H // 128   # 16 contraction tiles for gate/up
MTI = II // 128  # 32 I tiles
NMC = II // CHUNK  # 16 weight chunks
JJ = CHUNK // 128  # 2 m-tiles per chunk
KI = II // 128  # 32 contraction tiles for down
NH = H // 512   # 4 output column chunks


@functools.lru_cache(maxsize=2)
def _build(nb1: int):
    """Build the SPMD Bass program for capacity 512 + 128*nb1 tokens."""
    import concourse.mybir as mybir
    import concourse.tile as tile
    from concourse import bacc
    from concourse.masks import make_identity

    f32 = mybir.dt.float32
    f32r = mybir.dt.float32r
    NT = B0 // 128 + nb1

    nc = bacc.Bacc(None)
    xT0 = nc.declare_dram_parameter("xT0", [KT, 128, B0], f32r, isOutput=False)
    if nb1:
        xT1 = nc.declare_dram_parameter("xT1", [KT, 128, 128 * nb1], f32r, isOutput=False)
    wg = nc.declare_dram_parameter("wg", [H, II], f32r, isOutput=False)
    wu = nc.declare_dram_parameter("wu", [H, II], f32r, isOutput=False)
    wd = nc.declare_dram_parameter("wd", [II, H], f32r, isOutput=False)
    rw = nc.declare_dram_parameter("rw", [128, NT], f32, isOutput=False)
    dout = nc.declare_dram_parameter("d", [NT, 128, H], f32, isOutput=True)

    SILU = mybir.ActivationFunctionType.Silu

    with tile.TileContext(nc) as tc:
        with (
            tc.tile_pool(name="pers", bufs=1) as pers,
            tc.tile_pool(name="wpool", bufs=20) as wpool,
        ):
            aT0 = pers.tile([128, MTI, B0], f32r)
            if nb1:
                aT1 = pers.tile([128, nb1, MTI, 128], f32r)
                sg1 = pers.tile([128, nb1, NMC, CHUNK], f32)
                ident = pers.tile([128, 128], f32)
                make_identity(nc, ident[:])
            rwt = pers.tile([128, NT], f32)
            nc.gpsimd.dma_start(rwt[:], rw[:])

            with (
                tc.tile_pool(name="xp", bufs=1) as xp,
                tc.tile_pool(name="ps1", bufs=1, space="PSUM") as ps1,
                tc.tile_pool(name="sp", bufs=2) as sp,
            ):
                xt0 = []
                xt1 = []
                for k in range(KT):
                    t0 = xp.tile([128, B0], f32r, name=f"xt0_{k}")
                    nc.gpsimd.dma_start(t0[:], xT0[k])
                    xt0.append(t0)
                    if nb1:
                        t1 = xp.tile([128, 128 * nb1], f32r, name=f"xt1_{k}")
                        nc.gpsimd.dma_start(t1[:], xT1[k])
                        xt1.append(t1)

                for mc in range(NMC):
                    # One [128, 512] tile holds this chunk of BOTH w_gate
                    # (cols 0:256) and w_up (cols 256:512): block-1 then
                    # computes gate and up in a single full-rate N=512 matmul.
                    wb = []
                    for k in range(KT):
                        wbk = wpool.tile([128, 2 * CHUNK], f32r, name="wbk", tag="w")
                        nc.sync.dma_start(
                            wbk[:, :CHUNK],
                            wg[k * 128:(k + 1) * 128, mc * CHUNK:(mc + 1) * CHUNK],
                        )
                        nc.scalar.dma_start(
                            wbk[:, CHUNK:],
                            wu[k * 128:(k + 1) * 128, mc * CHUNK:(mc + 1) * CHUNK],
                        )
                        wb.append(wbk)

                    pg0 = [ps1.tile([128, B0], f32, name="pg0", tag="pg0", bufs=3)
                           for _ in range(JJ)]
                    pu0 = [ps1.tile([128, B0], f32, name="pu0", tag="pu0", bufs=3)
                           for _ in range(JJ)]
                    pgu1 = [ps1.tile([128, 2 * CHUNK], f32, name="pgu1", tag="pgu1", bufs=2)
                            for _ in range(nb1)]

                    for k in range(KT):
                        st = dict(start=(k == 0), stop=(k == KT - 1))
                        for j in range(JJ):
                            nc.tensor.matmul(
                                pg0[j][:], wb[k][:, j * 128:(j + 1) * 128], xt0[k][:], **st
                            )
                        for bt in range(nb1):
                            nc.tensor.matmul(
                                pgu1[bt][:], xt1[k][:, bt * 128:(bt + 1) * 128], wb[k][:], **st
                            )
                        for j in range(JJ):
                            nc.tensor.matmul(
                                pu0[j][:],
                                wb[k][:, CHUNK + j * 128:CHUNK + (j + 1) * 128],
                                xt0[k][:], **st
                            )

                    for j in range(JJ):
                        m = mc * JJ + j
                        sg = sp.tile([128, B0], f32, name="sg", tag="sg")
                        nc.scalar.activation(sg[:], pg0[j][:], SILU)
                        nc.vector.tensor_mul(aT0[:, m, :], sg[:], pu0[j][:])
                    for bt in range(nb1):
                        s1 = sg1[:, bt, mc, :]
                        nc.scalar.activation(s1, pgu1[bt][:, :CHUNK], SILU)
                        nc.vector.tensor_mul(s1, s1, pgu1[bt][:, CHUNK:])

            with (
                tc.tile_pool(name="ps2", bufs=1, space="PSUM") as ps2,
                tc.tile_pool(name="op", bufs=3) as op,
            ):
                pd_bufs = min(8, NT + 3)
                # Transpose the block-1 activations into [I, tokens] layout at
                # the head of phase 2 (the PE is otherwise idle while w_down
                # prefetch starts); copies alternate DVE/ACT to keep up.
                for bt in range(nb1):
                    for mi in range(MTI):
                        mc, t = divmod(mi, JJ)
                        ptr = ps2.tile([128, 128], f32, name="ptr", tag="pd", bufs=pd_bufs)
                        nc.tensor.transpose(
                            ptr[:], sg1[:, bt, mc, t * 128:(t + 1) * 128], ident[:]
                        )
                        if mi % 2 == 0:
                            nc.vector.tensor_copy(aT1[:, bt, mi, :], ptr[:])
                        else:
                            nc.scalar.copy(aT1[:, bt, mi, :], ptr[:])
                for nh in range(NH):
                    pd = [ps2.tile([128, 512], f32, name="pd", tag="pd", bufs=pd_bufs)
                          for _ in range(NT)]
                    for k in range(KI):
                        wdk = wpool.tile([128, 512], f32r, name="wdk", tag="w")
                        eng = nc.sync if k % 2 == 0 else nc.scalar
                        eng.dma_start(
                            wdk[:], wd[k * 128:(k + 1) * 128, nh * 512:(nh + 1) * 512]
                        )
                        st = dict(start=(k == 0), stop=(k == KI - 1))
                        for mt in range(NT):
                            if mt < B0 // 128:
                                lhsT = aT0[:, k, mt * 128:(mt + 1) * 128]
                            else:
                                lhsT = aT1[:, mt - B0 // 128, k, :]
                            nc.tensor.matmul(pd[mt][:], lhsT, wdk[:], **st)
                    for mt in range(NT):
                        ot = op.tile([128, 512], f32, name="ot", tag="ot")
                        nc.vector.tensor_scalar_mul(ot[:], pd[mt][:], rwt[:, mt:mt + 1])
                        nc.gpsimd.dma_start(dout[mt][:, nh * 512:(nh + 1) * 512], ot[:])

    nc.compile()
    return nc


@functools.lru_cache(maxsize=2)
def _get_exec(nb1: int):
    """Compile the Bass program and return (nc, run_fn) with a cached jit.

    run_fn(in_maps) -> list of per-core {"d": np.ndarray}. Mirrors
    bass2jax.run_bass_via_pjrt's multi-core branch, but keeps the jitted
    function alive across kernel() calls so repeat invocations skip XLA
    and NEFF compilation.
    """
    import jax
    import concourse.mybir as mybir
    from concourse import bass2jax

    nc = _build(nb1)
    bass2jax.install_neuronx_cc_hook()

    partition_name = nc.partition_id_tensor.name if nc.partition_id_tensor else None
    in_names, out_names, out_avals = [], [], []
    zero_out_shapes = []
    for alloc in nc.m.functions[0].allocations:
        if not isinstance(alloc, mybir.MemoryLocationSet):
            continue
        name = alloc.memorylocations[0].name
        if alloc.kind == "ExternalInput":
            if name != partition_name:
                in_names.append(name)
        elif alloc.kind == "ExternalOutput":
            shape = tuple(alloc.tensor_shape)
            dtype = mybir.dt.np(alloc.dtype)
            out_names.append(name)
            out_avals.append(jax.core.ShapedArray(shape, dtype))
            zero_out_shapes.append((shape, dtype))
    n_params = len(in_names)
    n_outs = len(out_names)
    all_names = list(in_names) + list(out_names)
    if partition_name is not None:
        all_names.append(partition_name)
    donate = tuple(range(n_params, n_params + n_outs))

    def _body(*args):
        operands = list(args)
        if partition_name is not None:
            operands.append(bass2jax.partition_id_tensor())
        outs = bass2jax._bass_exec_p.bind(
            *operands,
            out_avals=tuple(out_avals),
            in_names=tuple(all_names),
            out_names=tuple(out_names),
            lowering_input_output_aliases=(),
            sim_require_finite=True,
            sim_require_nnan=True,
            nc=nc,
        )
        return tuple(outs)

    devices = jax.devices()[:NCORES]
    assert len(devices) == NCORES, f"need {NCORES} devices, have {len(jax.devices())}"
    mesh = bass2jax.Mesh(np.asarray(devices), ("core",))
    in_specs = (bass2jax.PartitionSpec("core"),) * (n_params + n_outs)
    out_specs = (bass2jax.PartitionSpec("core"),) * n_outs
    sharded = jax.jit(
        bass2jax.shard_map(
            _body, mesh=mesh, in_specs=in_specs, out_specs=out_specs, check_rep=False
        ),
        donate_argnums=donate,
        keep_unused=True,
    )

    def run_fn(in_maps):
        concat_in = [
            np.concatenate([np.asarray(m[name]) for m in in_maps], axis=0)
            for name in in_names
        ]
        zeros = [
            np.zeros((shape[0] * NCORES,) + shape[1:], dtype)
            for shape, dtype in zero_out_shapes
        ]
        out_arrs = sharded(*concat_in, *zeros)
        results = []
        for c in range(NCORES):
            res = {}
            for i, name in enumerate(out_names):
                arr = np.asarray(out_arrs[i])
                per = arr.shape[0] // NCORES
                res[name] = arr[c * per:(c + 1) * per]
            results.append(res)
        return results

    return nc, run_fn


def _route(x, w_router):
    """Top-2 routing: expert ids + softmax weights, matching jax.lax.top_k
    (descending, ties to the lower index) + jax.nn.softmax."""
    logits = x.astype(np.float64) @ w_router.astype(np.float64)
    top2 = np.argsort(-logits, axis=1, kind="stable")[:, :TOPK]
    vals = np.take_along_axis(logits, top2, 1).astype(np.float32)
    e = np.exp(vals - vals.max(axis=1, keepdims=True))
    w = (e / e.sum(axis=1, keepdims=True)).astype(np.float32)
    return top2, w


def _reference_numpy(x, w_router, w_gate, w_up, w_down):
    """Correct-but-slow dense fallback for shapes the device program doesn't cover."""
    x = x.astype(np.float32)
    logits = x @ w_router.astype(np.float32)
    n_exp = w_gate.shape[0]
    k = min(TOPK, n_exp)
    top = np.argsort(-logits, axis=1, kind="stable")[:, :k]
    vals = np.take_along_axis(logits, top, 1)
    ex = np.exp(vals - vals.max(1, keepdims=True))
    ww = (ex / ex.sum(1, keepdims=True)).astype(np.float32)
    w_dense = np.zeros_like(logits)
    t_ids = np.arange(x.shape[0])[:, None]
    w_dense[t_ids, top] = ww
    out = np.zeros((x.shape[0], w_down.shape[-1]), np.float32)
    for e in range(n_exp):
        g = x @ w_gate[e]
        u = x @ w_up[e]
        a = (g / (1.0 + np.exp(-g))) * u
        out += w_dense[:, e:e + 1] * (a @ w_down[e])
    return out


def kernel(x, w_router, w_gate, w_up, w_down):
    x = np.ascontiguousarray(np.asarray(x, dtype=np.float32))
    w_router = np.asarray(w_router, dtype=np.float32)
    w_gate = np.ascontiguousarray(np.asarray(w_gate, dtype=np.float32))
    w_up = np.ascontiguousarray(np.asarray(w_up, dtype=np.float32))
    w_down = np.ascontiguousarray(np.asarray(w_down, dtype=np.float32))

    if (x.shape != (T, H) or w_router.shape != (H, E)
            or w_gate.shape != (E, H, II) or w_up.shape != (E, H, II)
            or w_down.shape != (E, II, H)):
        return _reference_numpy(x, w_router, w_gate, w_up, w_down)

    top2, w = _route(x, w_router)
    tok = np.repeat(np.arange(T), TOPK)
    te = top2.ravel()
    tw = w.ravel()
    toks_e, ws_e = [], []
    for e in range(E):
        sel = te == e
        toks_e.append(tok[sel])
        ws_e.append(tw[sel].astype(np.float32))

    max_n = max(len(t) for t in toks_e)
    nb1 = max(0, -(-(max_n - B0) // 128))  # ceil((max_n - 512)/128), >= 0
    if nb1 > 2:
        return _reference_numpy(x, w_router, w_gate, w_up, w_down)

    C = B0 + 128 * nb1
    NT = C // 128

    nc, run_fn = _get_exec(nb1)

    in_maps = []
    for e in range(E):
        n_e = len(toks_e[e])
        xg = np.zeros((C, H), np.float32)
        xg[:n_e] = x[toks_e[e]]
        xT = np.ascontiguousarray(xg.T).reshape(KT, 128, C)
        rfull = np.zeros(C, np.float32)
        rfull[:n_e] = ws_e[e]
        m = {
            "xT0": np.ascontiguousarray(xT[:, :, :B0]),
            "wg": w_gate[e],
            "wu": w_up[e],
            "wd": w_down[e],
            "rw": np.ascontiguousarray(rfull.reshape(NT, 128).T),
        }
        if nb1:
            m["xT1"] = np.ascontiguousarray(xT[:, :, B0:])
        in_maps.append(m)

    try:
        results = run_fn(in_maps)
    except Exception:
        import time as _time
        _time.sleep(20)
        results = run_fn(in_maps)

    out = np.zeros((T, H), np.float32)
    for e in range(E):
        n_e = len(toks_e[e])
        d = results[e]["d"].reshape(C, H)
        out[toks_e[e]] += d[:n_e]
    return out


# revision 15
# speedup vs baseline: 1.0494x; 1.0369x over previous
"""Trainium2 Bass kernel for nn_BlockSparseMLP (MoE gated MLP, E=8, top-2).

Strategy: expert parallelism over 8 NeuronCores. The router matmul
(x @ w_router, 67 MFLOP out of the 206 GFLOP total) plus the top-2
dispatch/gather and the final scatter-add combine run on the host; each
core runs the full gated MLP (gate/up, silu*up, down, weighted by the
routing prob) for the tokens routed to its expert. Matmuls use float32r
(full-rate fp32 mode on the PE, ~tf32 accuracy), activations stay fp32.

Per-core device layout (capacity C = 512 + 128*nb1 tokens, token-major
tiles of 128):
  phase 1 (gate/up): stream w_gate/w_up in [128, 256] chunks; block-0
    (512 tokens) uses weights as stationary and xT as moving (N=512);
    block-1 (the 128*nb1 overflow tokens) uses tokens as stationary and
    the weight chunk as moving (N=256), then PE-transposes the result
    into the [I, tokens] layout. silu(gate)*up is fused on ACT+DVE into
    aT ([I, C] layout, resident in SBUF).
  phase 2 (down): stream w_down in [128, 512] chunks (moving), aT tiles
    stationary, accumulate over I into [tokens, 512] psum tiles, scale
    by the per-token routing weight, DMA out.
"""

import sys
import functools

sys.path.insert(0, "/opt/trn_rl_repo")

import numpy as np

T, H, II, E, TOPK = 2048, 2048, 4096, 8, 2
NCORES = 8
B0 = 512        # block-0 token capacity (moving N for gate/up)
CHUNK = 256     # phase-1 weight chunk width along I
KT = H // 128   # 16 contraction tiles for gate/up
MTI = II // 128  # 32 I tiles
NMC = II // CHUNK  # 16 weight chunks
JJ = CHUNK // 128  # 2 m-tiles per chunk
KI = II // 128  # 32 contraction tiles for down
NH = H // 512   # 4 output column chunks


@functools.lru_cache(maxsize=2)
def _build(nb1: int):
    """Build the SPMD Bass program for capacity 512 + 128*nb1 tokens."""
    import concourse.mybir as mybir
    import concourse.tile as tile
    from concourse import bacc
    from concourse.masks import make_identity

    f32 = mybir.dt.float32
    f32r = mybir.dt.float32r
    NT = B0 // 128 + nb1

    nc = bacc.Bacc(None)
    xT0 = nc.declare_dram_parameter("xT0", [KT, 128, B0], f32r, isOutput=False)
    if nb1:
        xT1 = nc.declare_dram_parameter("xT1", [KT, 128, 128 * nb1], f32r, isOutput=False)
    wgu = nc.declare_dram_parameter("wgu", [NMC, KT, 128, 2 * CHUNK], f32r, isOutput=False)
    wd = nc.declare_dram_parameter("wd", [II, H], f32r, isOutput=False)
    rw = nc.declare_dram_parameter("rw", [128, NT], f32, isOutput=False)
    dout = nc.declare_dram_parameter("d", [NT, 128, H], f32, isOutput=True)

    SILU = mybir.ActivationFunctionType.Silu

    with tile.TileContext(nc) as tc:
        with (
            tc.tile_pool(name="pers", bufs=1) as pers,
            tc.tile_pool(name="wpool", bufs=20) as wpool,
        ):
            aT0 = pers.tile([128, MTI, B0], f32r)
            if nb1:
                aT1 = pers.tile([128, nb1, MTI, 128], f32r)
                sg1 = pers.tile([128, nb1, NMC, CHUNK], f32)
                ident = pers.tile([128, 128], f32)
                make_identity(nc, ident[:])
            rwt = pers.tile([128, NT], f32)
            nc.gpsimd.dma_start(rwt[:], rw[:])

            with (
                tc.tile_pool(name="xp", bufs=1) as xp,
                tc.tile_pool(name="ps1", bufs=1, space="PSUM") as ps1,
                tc.tile_pool(name="sp", bufs=2) as sp,
            ):
                xt0 = []
                xt1 = []
                for k in range(KT):
                    eng = nc.sync if k % 2 == 0 else nc.scalar
                    t0 = xp.tile([128, B0], f32r, name=f"xt0_{k}")
                    eng.dma_start(t0[:], xT0[k])
                    xt0.append(t0)
                    if nb1:
                        t1 = xp.tile([128, 128 * nb1], f32r, name=f"xt1_{k}")
                        eng.dma_start(t1[:], xT1[k])
                        xt1.append(t1)

                for mc in range(NMC):
                    # One [128, 512] tile holds this chunk of BOTH w_gate
                    # (cols 0:256) and w_up (cols 256:512): block-1 then
                    # computes gate and up in a single full-rate N=512 matmul.
                    wb = []
                    for k in range(KT):
                        wbk = wpool.tile([128, 2 * CHUNK], f32r, name="wbk", tag="w")
                        eng = nc.sync if k % 2 == 0 else nc.scalar
                        eng.dma_start(wbk[:], wgu[mc, k])
                        wb.append(wbk)

                    pg0 = [ps1.tile([128, B0], f32, name="pg0", tag="pg0", bufs=3)
                           for _ in range(JJ)]
                    pu0 = [ps1.tile([128, B0], f32, name="pu0", tag="pu0", bufs=3)
                           for _ in range(JJ)]
                    pgu1 = [ps1.tile([128, 2 * CHUNK], f32, name="pgu1", tag="pgu1", bufs=2)
                            for _ in range(nb1)]

                    for k in range(KT):
                        st = dict(start=(k == 0), stop=(k == KT - 1))
                        for j in range(JJ):
                            nc.tensor.matmul(
                                pg0[j][:], wb[k][:, j * 128:(j + 1) * 128], xt0[k][:], **st
                            )
                        for bt in range(nb1):
                            nc.tensor.matmul(
                                pgu1[bt][:], xt1[k][:, bt * 128:(bt + 1) * 128], wb[k][:], **st
                            )
                        for j in range(JJ):
                            nc.tensor.matmul(
                                pu0[j][:],
                                wb[k][:, CHUNK + j * 128:CHUNK + (j + 1) * 128],
                                xt0[k][:], **st
                            )

                    for j in range(JJ):
                        m = mc * JJ + j
                        sg = sp.tile([128, B0], f32, name="sg", tag="sg")
                        nc.scalar.activation(sg[:], pg0[j][:], SILU)
                        nc.vector.tensor_mul(aT0[:, m, :], sg[:], pu0[j][:])
                    for bt in range(nb1):
                        s1 = sg1[:, bt, mc, :]
                        nc.scalar.activation(s1, pgu1[bt][:, :CHUNK], SILU)
                        nc.vector.tensor_mul(s1, s1, pgu1[bt][:, CHUNK:])

            with (
                tc.tile_pool(name="ps2", bufs=1, space="PSUM") as ps2,
                tc.tile_pool(name="op", bufs=3) as op,
            ):
                pd_bufs = min(8, NT + 3)
                # Transpose the block-1 activations into [I, tokens] layout at
                # the head of phase 2 (the PE is otherwise idle while w_down
                # prefetch starts); copies alternate DVE/ACT to keep up.
                for bt in range(nb1):
                    for mi in range(MTI):
                        mc, t = divmod(mi, JJ)
                        ptr = ps2.tile([128, 128], f32, name="ptr", tag="pd", bufs=pd_bufs)
                        nc.tensor.transpose(
                            ptr[:], sg1[:, bt, mc, t * 128:(t + 1) * 128], ident[:]
                        )
                        if mi % 2 == 0:
                            nc.vector.tensor_copy(aT1[:, bt, mi, :], ptr[:])
                        else:
                            nc.scalar.copy(aT1[:, bt, mi, :], ptr[:])
                for nh in range(NH):
                    pd = [ps2.tile([128, 512], f32, name="pd", tag="pd", bufs=pd_bufs)
                          for _ in range(NT)]
                    for k in range(KI):
                        wdk = wpool.tile([128, 512], f32r, name="wdk", tag="w")
                        eng = nc.sync if k % 2 == 0 else nc.scalar
                        eng.dma_start(
                            wdk[:], wd[k * 128:(k + 1) * 128, nh * 512:(nh + 1) * 512]
                        )
                        st = dict(start=(k == 0), stop=(k == KI - 1))
                        for mt in range(NT):
                            if mt < B0 // 128:
                                lhsT = aT0[:, k, mt * 128:(mt + 1) * 128]
                            else:
                                lhsT = aT1[:, mt - B0 // 128, k, :]
                            nc.tensor.matmul(pd[mt][:], lhsT, wdk[:], **st)
                    for mt in range(NT):
                        ot = op.tile([128, 512], f32, name="ot", tag="ot")
                        nc.vector.tensor_scalar_mul(ot[:], pd[mt][:], rwt[:, mt:mt + 1])
                        nc.gpsimd.dma_start(dout[mt][:, nh * 512:(nh + 1) * 512], ot[:])

    nc.compile()
    return nc


@functools.lru_cache(maxsize=2)
def _get_exec(nb1: int):
    """Compile the Bass program and return (nc, run_fn) with a cached jit.

    run_fn(in_maps) -> list of per-core {"d": np.ndarray}. Mirrors
    bass2jax.run_bass_via_pjrt's multi-core branch, but keeps the jitted
    function alive across kernel() calls so repeat invocations skip XLA
    and NEFF compilation.
    """
    import jax
    import concourse.mybir as mybir
    from concourse import bass2jax

    nc = _build(nb1)
    bass2jax.install_neuronx_cc_hook()

    partition_name = nc.partition_id_tensor.name if nc.partition_id_tensor else None
    in_names, out_names, out_avals = [], [], []
    zero_out_shapes = []
    for alloc in nc.m.functions[0].allocations:
        if not isinstance(alloc, mybir.MemoryLocationSet):
            continue
        name = alloc.memorylocations[0].name
        if alloc.kind == "ExternalInput":
            if name != partition_name:
                in_names.append(name)
        elif alloc.kind == "ExternalOutput":
            shape = tuple(alloc.tensor_shape)
            dtype = mybir.dt.np(alloc.dtype)
            out_names.append(name)
            out_avals.append(jax.core.ShapedArray(shape, dtype))
            zero_out_shapes.append((shape, dtype))
    n_params = len(in_names)
    n_outs = len(out_names)
    all_names = list(in_names) + list(out_names)
    if partition_name is not None:
        all_names.append(partition_name)
    donate = tuple(range(n_params, n_params + n_outs))

    def _body(*args):
        operands = list(args)
        if partition_name is not None:
            operands.append(bass2jax.partition_id_tensor())
        outs = bass2jax._bass_exec_p.bind(
            *operands,
            out_avals=tuple(out_avals),
            in_names=tuple(all_names),
            out_names=tuple(out_names),
            lowering_input_output_aliases=(),
            sim_require_finite=True,
            sim_require_nnan=True,
            nc=nc,
        )
        return tuple(outs)

    devices = jax.devices()[:NCORES]
    assert len(devices) == NCORES, f"need {NCORES} devices, have {len(jax.devices())}"
    mesh = bass2jax.Mesh(np.asarray(devices), ("core",))
    in_specs = (bass2jax.PartitionSpec("core"),) * (n_params + n_outs)
    out_specs = (bass2jax.PartitionSpec("core"),) * n_outs
    sharded = jax.jit(
        bass2jax.shard_map(
            _body, mesh=mesh, in_specs=in_specs, out_specs=out_specs, check_rep=False
        ),
        donate_argnums=donate,
        keep_unused=True,
    )

    def run_fn(in_maps):
        concat_in = [
            np.concatenate([np.asarray(m[name]) for m in in_maps], axis=0)
            for name in in_names
        ]
        zeros = [
            np.zeros((shape[0] * NCORES,) + shape[1:], dtype)
            for shape, dtype in zero_out_shapes
        ]
        out_arrs = sharded(*concat_in, *zeros)
        results = []
        for c in range(NCORES):
            res = {}
            for i, name in enumerate(out_names):
                arr = np.asarray(out_arrs[i])
                per = arr.shape[0] // NCORES
                res[name] = arr[c * per:(c + 1) * per]
            results.append(res)
        return results

    return nc, run_fn


def _route(x, w_router):
    """Top-2 routing: expert ids + softmax weights, matching jax.lax.top_k
    (descending, ties to the lower index) + jax.nn.softmax."""
    logits = x.astype(np.float64) @ w_router.astype(np.float64)
    top2 = np.argsort(-logits, axis=1, kind="stable")[:, :TOPK]
    vals = np.take_along_axis(logits, top2, 1).astype(np.float32)
    e = np.exp(vals - vals.max(axis=1, keepdims=True))
    w = (e / e.sum(axis=1, keepdims=True)).astype(np.float32)
    return top2, w


def _reference_numpy(x, w_router, w_gate, w_up, w_down):
    """Correct-but-slow dense fallback for shapes the device program doesn't cover."""
    x = x.astype(np.float32)
    logits = x @ w_router.astype(np.float32)
    n_exp = w_gate.shape[0]
    k = min(TOPK, n_exp)
    top = np.argsort(-logits, axis=1, kind="stable")[:, :k]
    vals = np.take_along_axis(logits, top, 1)
    ex = np.exp(vals - vals.max(1, keepdims=True))
    ww = (ex / ex.sum(1, keepdims=True)).astype(np.float32)
    w_dense = np.zeros_like(logits)
    t_ids = np.arange(x.shape[0])[:, None]
    w_dense[t_ids, top] = ww
    out = np.zeros((x.shape[0], w_down.shape[-1]), np.float32)
    for e in range(n_exp):
        g = x @ w_gate[e]
        u = x @ w_up[e]
        a = (g / (1.0 + np.exp(-g))) * u
        out += w_dense[:, e:e + 1] * (a @ w_down[e])
    return out


def _pack_core_inputs(x, wg_e, wu_e, wd_e, toks, ws, nb1):
    """Build one core's input map: gathered/transposed tokens, packed
    gate|up weight tiles ([NMC, KT, 128, 512] matching the SBUF layout so
    each tile is one contiguous 2KB-run DMA), routing weights."""
    C = B0 + 128 * nb1
    NT = C // 128
    n_e = len(toks)
    xg = np.zeros((C, H), np.float32)
    xg[:n_e] = x[toks]
    xT = np.ascontiguousarray(xg.T).reshape(KT, 128, C)
    rfull = np.zeros(C, np.float32)
    rfull[:n_e] = ws
    wgu = np.empty((NMC, KT, 128, 2 * CHUNK), np.float32)
    wgu[..., :CHUNK] = wg_e.reshape(KT, 128, NMC, CHUNK).transpose(2, 0, 1, 3)
    wgu[..., CHUNK:] = wu_e.reshape(KT, 128, NMC, CHUNK).transpose(2, 0, 1, 3)
    m = {
        "xT0": np.ascontiguousarray(xT[:, :, :B0]),
        "wgu": wgu,
        "wd": wd_e,
        "rw": np.ascontiguousarray(rfull.reshape(NT, 128).T),
    }
    if nb1:
        m["xT1"] = np.ascontiguousarray(xT[:, :, B0:])
    return m


def kernel(x, w_router, w_gate, w_up, w_down):
    x = np.ascontiguousarray(np.asarray(x, dtype=np.float32))
    w_router = np.asarray(w_router, dtype=np.float32)
    w_gate = np.ascontiguousarray(np.asarray(w_gate, dtype=np.float32))
    w_up = np.ascontiguousarray(np.asarray(w_up, dtype=np.float32))
    w_down = np.ascontiguousarray(np.asarray(w_down, dtype=np.float32))

    if (x.shape != (T, H) or w_router.shape != (H, E)
            or w_gate.shape != (E, H, II) or w_up.shape != (E, H, II)
            or w_down.shape != (E, II, H)):
        return _reference_numpy(x, w_router, w_gate, w_up, w_down)

    top2, w = _route(x, w_router)
    tok = np.repeat(np.arange(T), TOPK)
    te = top2.ravel()
    tw = w.ravel()
    toks_e, ws_e = [], []
    for e in range(E):
        sel = te == e
        toks_e.append(tok[sel])
        ws_e.append(tw[sel].astype(np.float32))

    max_n = max(len(t) for t in toks_e)
    nb1 = max(0, -(-(max_n - B0) // 128))  # ceil((max_n - 512)/128), >= 0
    if nb1 > 2:
        return _reference_numpy(x, w_router, w_gate, w_up, w_down)

    C = B0 + 128 * nb1
    NT = C // 128

    nc, run_fn = _get_exec(nb1)

    in_maps = [
        _pack_core_inputs(x, w_gate[e], w_up[e], w_down[e], toks_e[e], ws_e[e], nb1)
        for e in range(E)
    ]

    try:
        results = run_fn(in_maps)
    except Exception:
        import time as _time
        _time.sleep(20)
        results = run_fn(in_maps)

    out = np.zeros((T, H), np.float32)
    for e in range(E):
        n_e = len(toks_e[e])
        d = results[e]["d"].reshape(C, H)
        out[toks_e[e]] += d[:n_e]
    return out


# revision 16
# speedup vs baseline: 1.0586x; 1.0088x over previous
"""Trainium2 Bass kernel for nn_BlockSparseMLP (MoE gated MLP, E=8, top-2).

Strategy: expert parallelism over 8 NeuronCores. The router matmul
(x @ w_router, 67 MFLOP out of the 206 GFLOP total) plus the top-2
dispatch/gather and the final scatter-add combine run on the host; each
core runs the full gated MLP (gate/up, silu*up, down, weighted by the
routing prob) for the tokens routed to its expert. Matmuls use float32r
(full-rate fp32 mode on the PE, ~tf32 accuracy), activations stay fp32.

Per-core device layout (capacity C = 512 + 128*nb1 tokens, token-major
tiles of 128):
  phase 1 (gate/up): stream w_gate/w_up in [128, 256] chunks; block-0
    (512 tokens) uses weights as stationary and xT as moving (N=512);
    block-1 (the 128*nb1 overflow tokens) uses tokens as stationary and
    the weight chunk as moving (N=256), then PE-transposes the result
    into the [I, tokens] layout. silu(gate)*up is fused on ACT+DVE into
    aT ([I, C] layout, resident in SBUF).
  phase 2 (down): stream w_down in [128, 512] chunks (moving), aT tiles
    stationary, accumulate over I into [tokens, 512] psum tiles, scale
    by the per-token routing weight, DMA out.
"""

import sys
import functools

sys.path.insert(0, "/opt/trn_rl_repo")

import numpy as np

T, H, II, E, TOPK = 2048, 2048, 4096, 8, 2
NCORES = 8
B0 = 512        # block-0 token capacity (moving N for gate/up)
CHUNK = 256     # phase-1 weight chunk width along I
KT = H // 128   # 16 contraction tiles for gate/up
MTI = II // 128  # 32 I tiles
NMC = II // CHUNK  # 16 weight chunks
JJ = CHUNK // 128  # 2 m-tiles per chunk
KI = II // 128  # 32 contraction tiles for down
NH = H // 512   # 4 output column chunks


@functools.lru_cache(maxsize=2)
def _build(nb1: int):
    """Build the SPMD Bass program for capacity 512 + 128*nb1 tokens."""
    import concourse.mybir as mybir
    import concourse.tile as tile
    from concourse import bacc
    from concourse.masks import make_identity

    f32 = mybir.dt.float32
    f32r = mybir.dt.float32r
    NT = B0 // 128 + nb1

    nc = bacc.Bacc(None)
    xT0 = nc.declare_dram_parameter("xT0", [KT, 128, B0], f32r, isOutput=False)
    if nb1:
        xT1 = nc.declare_dram_parameter("xT1", [KT, 128, 128 * nb1], f32r, isOutput=False)
    wgu = nc.declare_dram_parameter("wgu", [NMC, KT, 128, 2 * CHUNK], f32r, isOutput=False)
    wd = nc.declare_dram_parameter("wd", [II, H], f32r, isOutput=False)
    rw = nc.declare_dram_parameter("rw", [128, NT], f32, isOutput=False)
    dout = nc.declare_dram_parameter("d", [NT, 128, H], f32, isOutput=True)

    SILU = mybir.ActivationFunctionType.Silu

    with tile.TileContext(nc) as tc:
        with (
            tc.tile_pool(name="pers", bufs=1) as pers,
            tc.tile_pool(name="wpool", bufs=20) as wpool,
        ):
            aT0 = pers.tile([128, MTI, B0], f32r)
            if nb1:
                aT1 = pers.tile([128, nb1, MTI, 128], f32r)
                sg1 = pers.tile([128, nb1, NMC, CHUNK], f32)
                ident = pers.tile([128, 128], f32)
                make_identity(nc, ident[:])
            rwt = pers.tile([128, NT], f32)
            nc.gpsimd.dma_start(rwt[:], rw[:])

            with (
                tc.tile_pool(name="xp", bufs=1) as xp,
                tc.tile_pool(name="ps1", bufs=1, space="PSUM") as ps1,
                tc.tile_pool(name="sp", bufs=2) as sp,
            ):
                xt0 = [xp.tile([128, B0], f32r, name=f"xt0_{k}") for k in range(KT)]
                xt1 = [xp.tile([128, 128 * nb1], f32r, name=f"xt1_{k}")
                       for k in range(KT)] if nb1 else []

                for mc in range(NMC):
                    # One [128, 512] tile holds this chunk of BOTH w_gate
                    # (cols 0:256) and w_up (cols 256:512): block-1 then
                    # computes gate and up in a single full-rate N=512 matmul.
                    # The xT loads are interleaved with mc0's weight loads on
                    # the opposite HWDGE queue so the k-loop's inputs arrive
                    # in consumption order instead of serializing the head.
                    wb = []
                    for k in range(KT):
                        eng_x = nc.sync if k % 2 == 0 else nc.scalar
                        eng_w = nc.scalar if k % 2 == 0 else nc.sync
                        if mc == 0:
                            eng_x.dma_start(xt0[k][:], xT0[k])
                            if nb1:
                                eng_x.dma_start(xt1[k][:], xT1[k])
                        wbk = wpool.tile([128, 2 * CHUNK], f32r, name="wbk", tag="w")
                        eng_w.dma_start(wbk[:], wgu[mc, k])
                        wb.append(wbk)

                    pg0 = [ps1.tile([128, B0], f32, name="pg0", tag="pg0", bufs=3)
                           for _ in range(JJ)]
                    pu0 = [ps1.tile([128, B0], f32, name="pu0", tag="pu0", bufs=3)
                           for _ in range(JJ)]
                    pgu1 = [ps1.tile([128, 2 * CHUNK], f32, name="pgu1", tag="pgu1", bufs=2)
                            for _ in range(nb1)]

                    for k in range(KT):
                        st = dict(start=(k == 0), stop=(k == KT - 1))
                        for j in range(JJ):
                            nc.tensor.matmul(
                                pg0[j][:], wb[k][:, j * 128:(j + 1) * 128], xt0[k][:], **st
                            )
                        for bt in range(nb1):
                            nc.tensor.matmul(
                                pgu1[bt][:], xt1[k][:, bt * 128:(bt + 1) * 128], wb[k][:], **st
                            )
                        for j in range(JJ):
                            nc.tensor.matmul(
                                pu0[j][:],
                                wb[k][:, CHUNK + j * 128:CHUNK + (j + 1) * 128],
                                xt0[k][:], **st
                            )

                    for j in range(JJ):
                        m = mc * JJ + j
                        sg = sp.tile([128, B0], f32, name="sg", tag="sg")
                        nc.scalar.activation(sg[:], pg0[j][:], SILU)
                        nc.vector.tensor_mul(aT0[:, m, :], sg[:], pu0[j][:])
                    for bt in range(nb1):
                        s1 = sg1[:, bt, mc, :]
                        nc.scalar.activation(s1, pgu1[bt][:, :CHUNK], SILU)
                        nc.vector.tensor_mul(s1, s1, pgu1[bt][:, CHUNK:])

            with (
                tc.tile_pool(name="ps2", bufs=1, space="PSUM") as ps2,
                tc.tile_pool(name="op", bufs=3) as op,
            ):
                pd_bufs = min(8, NT + 3)
                # Transpose the block-1 activations into [I, tokens] layout at
                # the head of phase 2 (the PE is otherwise idle while w_down
                # prefetch starts); copies alternate DVE/ACT to keep up.
                for bt in range(nb1):
                    for mi in range(MTI):
                        mc, t = divmod(mi, JJ)
                        ptr = ps2.tile([128, 128], f32, name="ptr", tag="pd", bufs=pd_bufs)
                        nc.tensor.transpose(
                            ptr[:], sg1[:, bt, mc, t * 128:(t + 1) * 128], ident[:]
                        )
                        if mi % 2 == 0:
                            nc.vector.tensor_copy(aT1[:, bt, mi, :], ptr[:])
                        else:
                            nc.scalar.copy(aT1[:, bt, mi, :], ptr[:])
                for nh in range(NH):
                    pd = [ps2.tile([128, 512], f32, name="pd", tag="pd", bufs=pd_bufs)
                          for _ in range(NT)]
                    for k in range(KI):
                        wdk = wpool.tile([128, 512], f32r, name="wdk", tag="w")
                        eng = nc.sync if k % 2 == 0 else nc.scalar
                        eng.dma_start(
                            wdk[:], wd[k * 128:(k + 1) * 128, nh * 512:(nh + 1) * 512]
                        )
                        st = dict(start=(k == 0), stop=(k == KI - 1))
                        for mt in range(NT):
                            if mt < B0 // 128:
                                lhsT = aT0[:, k, mt * 128:(mt + 1) * 128]
                            else:
                                lhsT = aT1[:, mt - B0 // 128, k, :]
                            nc.tensor.matmul(pd[mt][:], lhsT, wdk[:], **st)
                    for mt in range(NT):
                        ot = op.tile([128, 512], f32, name="ot", tag="ot")
                        nc.vector.tensor_scalar_mul(ot[:], pd[mt][:], rwt[:, mt:mt + 1])
                        nc.gpsimd.dma_start(dout[mt][:, nh * 512:(nh + 1) * 512], ot[:])

    nc.compile()
    return nc


@functools.lru_cache(maxsize=2)
def _get_exec(nb1: int):
    """Compile the Bass program and return (nc, run_fn) with a cached jit.

    run_fn(in_maps) -> list of per-core {"d": np.ndarray}. Mirrors
    bass2jax.run_bass_via_pjrt's multi-core branch, but keeps the jitted
    function alive across kernel() calls so repeat invocations skip XLA
    and NEFF compilation.
    """
    import jax
    import concourse.mybir as mybir
    from concourse import bass2jax

    nc = _build(nb1)
    bass2jax.install_neuronx_cc_hook()

    partition_name = nc.partition_id_tensor.name if nc.partition_id_tensor else None
    in_names, out_names, out_avals = [], [], []
    zero_out_shapes = []
    for alloc in nc.m.functions[0].allocations:
        if not isinstance(alloc, mybir.MemoryLocationSet):
            continue
        name = alloc.memorylocations[0].name
        if alloc.kind == "ExternalInput":
            if name != partition_name:
                in_names.append(name)
        elif alloc.kind == "ExternalOutput":
            shape = tuple(alloc.tensor_shape)
            dtype = mybir.dt.np(alloc.dtype)
            out_names.append(name)
            out_avals.append(jax.core.ShapedArray(shape, dtype))
            zero_out_shapes.append((shape, dtype))
    n_params = len(in_names)
    n_outs = len(out_names)
    all_names = list(in_names) + list(out_names)
    if partition_name is not None:
        all_names.append(partition_name)
    donate = tuple(range(n_params, n_params + n_outs))

    def _body(*args):
        operands = list(args)
        if partition_name is not None:
            operands.append(bass2jax.partition_id_tensor())
        outs = bass2jax._bass_exec_p.bind(
            *operands,
            out_avals=tuple(out_avals),
            in_names=tuple(all_names),
            out_names=tuple(out_names),
            lowering_input_output_aliases=(),
            sim_require_finite=True,
            sim_require_nnan=True,
            nc=nc,
        )
        return tuple(outs)

    devices = jax.devices()[:NCORES]
    assert len(devices) == NCORES, f"need {NCORES} devices, have {len(jax.devices())}"
    mesh = bass2jax.Mesh(np.asarray(devices), ("core",))
    in_specs = (bass2jax.PartitionSpec("core"),) * (n_params + n_outs)
    out_specs = (bass2jax.PartitionSpec("core"),) * n_outs
    sharded = jax.jit(
        bass2jax.shard_map(
            _body, mesh=mesh, in_specs=in_specs, out_specs=out_specs, check_rep=False
        ),
        donate_argnums=donate,
        keep_unused=True,
    )

    def run_fn(in_maps):
        concat_in = [
            np.concatenate([np.asarray(m[name]) for m in in_maps], axis=0)
            for name in in_names
        ]
        zeros = [
            np.zeros((shape[0] * NCORES,) + shape[1:], dtype)
            for shape, dtype in zero_out_shapes
        ]
        out_arrs = sharded(*concat_in, *zeros)
        results = []
        for c in range(NCORES):
            res = {}
            for i, name in enumerate(out_names):
                arr = np.asarray(out_arrs[i])
                per = arr.shape[0] // NCORES
                res[name] = arr[c * per:(c + 1) * per]
            results.append(res)
        return results

    return nc, run_fn


def _route(x, w_router):
    """Top-2 routing: expert ids + softmax weights, matching jax.lax.top_k
    (descending, ties to the lower index) + jax.nn.softmax."""
    logits = x.astype(np.float64) @ w_router.astype(np.float64)
    top2 = np.argsort(-logits, axis=1, kind="stable")[:, :TOPK]
    vals = np.take_along_axis(logits, top2, 1).astype(np.float32)
    e = np.exp(vals - vals.max(axis=1, keepdims=True))
    w = (e / e.sum(axis=1, keepdims=True)).astype(np.float32)
    return top2, w


def _reference_numpy(x, w_router, w_gate, w_up, w_down):
    """Correct-but-slow dense fallback for shapes the device program doesn't cover."""
    x = x.astype(np.float32)
    logits = x @ w_router.astype(np.float32)
    n_exp = w_gate.shape[0]
    k = min(TOPK, n_exp)
    top = np.argsort(-logits, axis=1, kind="stable")[:, :k]
    vals = np.take_along_axis(logits, top, 1)
    ex = np.exp(vals - vals.max(1, keepdims=True))
    ww = (ex / ex.sum(1, keepdims=True)).astype(np.float32)
    w_dense = np.zeros_like(logits)
    t_ids = np.arange(x.shape[0])[:, None]
    w_dense[t_ids, top] = ww
    out = np.zeros((x.shape[0], w_down.shape[-1]), np.float32)
    for e in range(n_exp):
        g = x @ w_gate[e]
        u = x @ w_up[e]
        a = (g / (1.0 + np.exp(-g))) * u
        out += w_dense[:, e:e + 1] * (a @ w_down[e])
    return out


def _pack_core_inputs(x, wg_e, wu_e, wd_e, toks, ws, nb1):
    """Build one core's input map: gathered/transposed tokens, packed
    gate|up weight tiles ([NMC, KT, 128, 512] matching the SBUF layout so
    each tile is one contiguous 2KB-run DMA), routing weights."""
    C = B0 + 128 * nb1
    NT = C // 128
    n_e = len(toks)
    xg = np.zeros((C, H), np.float32)
    xg[:n_e] = x[toks]
    xT = np.ascontiguousarray(xg.T).reshape(KT, 128, C)
    rfull = np.zeros(C, np.float32)
    rfull[:n_e] = ws
    wgu = np.empty((NMC, KT, 128, 2 * CHUNK), np.float32)
    wgu[..., :CHUNK] = wg_e.reshape(KT, 128, NMC, CHUNK).transpose(2, 0, 1, 3)
    wgu[..., CHUNK:] = wu_e.reshape(KT, 128, NMC, CHUNK).transpose(2, 0, 1, 3)
    m = {
        "xT0": np.ascontiguousarray(xT[:, :, :B0]),
        "wgu": wgu,
        "wd": wd_e,
        "rw": np.ascontiguousarray(rfull.reshape(NT, 128).T),
    }
    if nb1:
        m["xT1"] = np.ascontiguousarray(xT[:, :, B0:])
    return m


def kernel(x, w_router, w_gate, w_up, w_down):
    x = np.ascontiguousarray(np.asarray(x, dtype=np.float32))
    w_router = np.asarray(w_router, dtype=np.float32)
    w_gate = np.ascontiguousarray(np.asarray(w_gate, dtype=np.float32))
    w_up = np.ascontiguousarray(np.asarray(w_up, dtype=np.float32))
    w_down = np.ascontiguousarray(np.asarray(w_down, dtype=np.float32))

    if (x.shape != (T, H) or w_router.shape != (H, E)
            or w_gate.shape != (E, H, II) or w_up.shape != (E, H, II)
            or w_down.shape != (E, II, H)):
        return _reference_numpy(x, w_router, w_gate, w_up, w_down)

    top2, w = _route(x, w_router)
    tok = np.repeat(np.arange(T), TOPK)
    te = top2.ravel()
    tw = w.ravel()
    toks_e, ws_e = [], []
    for e in range(E):
        sel = te == e
        toks_e.append(tok[sel])
        ws_e.append(tw[sel].astype(np.float32))

    max_n = max(len(t) for t in toks_e)
    nb1 = max(0, -(-(max_n - B0) // 128))  # ceil((max_n - 512)/128), >= 0
    if nb1 > 2:
        return _reference_numpy(x, w_router, w_gate, w_up, w_down)

    C = B0 + 128 * nb1
    NT = C // 128

    nc, run_fn = _get_exec(nb1)

    in_maps = [
        _pack_core_inputs(x, w_gate[e], w_up[e], w_down[e], toks_e[e], ws_e[e], nb1)
        for e in range(E)
    ]

    try:
        results = run_fn(in_maps)
    except Exception:
        import time as _time
        _time.sleep(20)
        results = run_fn(in_maps)

    out = np.zeros((T, H), np.float32)
    for e in range(E):
        n_e = len(toks_e[e])
        d = results[e]["d"].reshape(C, H)
        out[toks_e[e]] += d[:n_e]
    return out
